# revision 12
# baseline (speedup 1.0000x reference)
"""CAM (channel self-attention) kernel for Trainium2 — 8 NeuronCores, batch-parallel.

Math per batch element b (A = x[b] reshaped [N=4096, C=512]):
    G = A^T A                  [C, C]
    P = softmax_rows(G)        [C, C]
    Y = A P                    [N, C]
    out = gamma * Y + x

Sharding: data-parallel over batch — core i handles batch element i.
No cross-core communication needed.

Design notes:
  - fp8e4 (E4M3) operands with MatmulPerfMode.DoubleRow: each matmul
    contracts TWO 128-row k-tiles per pass at ~0.5 cycles/output-row.
    Accumulation stays f32 in PSUM; the f32 epilogue (gamma * Y + x)
    keeps the residual path exact.
  - Partition-contiguous layout: x viewed as [128, 32, 512] via
    "(p t) c" — partition p holds rows 32p..32p+31, so each DMA group
    moves large contiguous runs per partition (input and output phases
    run at the HBM roofline).
  - Per chunk-pair emission order: casts -> Gram matmuls -> transposes,
    so the last Gram (which gates softmax) never queues behind the
    transpose backlog; late transposes overlap softmax (Y only needs
    chunk t's transpose at its own output chunk).
  - Full Gram accumulated in 4 PSUM banks; softmax reads straight from
    PSUM. One shared 6-buffer PSUM pool rotates warm -> g0..g3 -> y
    tiles, so the Y phase gets a ~6-deep bank pipeline (the g banks are
    recycled after their softmax reads complete).
  - A^T stored stride-2 padded ([...,128,2] fp8) matching the fp8 PE
    transpose output step; PSUM->SBUF copies then run as contiguous
    16-bit moves on ACT.
  - Softmax: DVE row-max (negated, from PSUM) -> ACT exp with fused
    row-sum -> DVE reciprocal -> DVE per-row scale to fp8 P.
  - Epilogue: DVE scalar_tensor_tensor out = (Y * gamma) + x, staged in
    1 MiB groups, DMA'd out with 8KB/partition descriptors.
"""

import numpy as np

import concourse.tile as tile
from concourse import bacc, mybir
from concourse.bass_utils import run_bass_kernel_spmd
from concourse.masks import make_identity

B = 8
H = 64
W = 64
C = 512
HW = H * W            # 4096 rows per batch element
NT = HW // 128        # 32 row chunks of 128 (chunk k = rows {32p + k})
CT = C // 128         # 4

F32 = mybir.dt.float32
U16 = mybir.dt.uint16
FP8 = mybir.dt.float8e4
DR = mybir.MatmulPerfMode.DoubleRow

_CACHE = {}


def _emit(nc, tc, out, x, gamma):
    from contextlib import ExitStack

    with ExitStack() as ctx:
        big = ctx.enter_context(tc.tile_pool(name="big", bufs=1))
        small = ctx.enter_context(tc.tile_pool(name="small", bufs=1))
        stat = ctx.enter_context(tc.tile_pool(name="stat", bufs=4))
        ostage = ctx.enter_context(tc.tile_pool(name="ostage", bufs=4))
        ps = ctx.enter_context(tc.tile_pool(name="ps", bufs=6, space="PSUM"))
        tps = ctx.enter_context(tc.tile_pool(name="tps", bufs=2, space="PSUM"))

        A32 = big.tile([128, NT, C], F32)       # x rows, row 32p+t on part p
        A8 = big.tile([128, NT, C], FP8)        # fp8 cast of A32
        # A^T, stride-2 padded (fp8 PE transposes write with element step 2;
        # keeping the pad lets the PSUM->SBUF copy run as contiguous u16):
        # AT8[p, ci, k, j, 0] = A[32j+k, 128ci+p]
        AT8 = big.tile([128, CT, NT, 128, 2], FP8)
        E32 = big.tile([128, CT, C], F32)       # exp(G - rowmax)
        P8 = big.tile([128, CT, C], FP8)        # softmax(G) in fp8

        ident8 = small.tile([128, 128], FP8)
        make_identity(nc, ident8[:])

        gB = small.tile([128, 1], F32)          # gamma broadcast to partitions

        # PE warm-up: HAM clock gate holds the PE at 1.2 GHz until it has
        # been busy a while; burn the DMA lead-in with short dummy DoubleRow
        # matmuls (kept brief so chunk 0's real work isn't queued behind it).
        warm8 = small.tile([128, 2, C], FP8)
        nc.gpsimd.memset(warm8[:], 0.0)
        warm_ps = ps.tile([128, C], F32, name="ps", tag="ps")
        NW = 8
        for wi in range(NW):
            nc.tensor.matmul(
                warm_ps[:, 0:256], warm8[:, :, 0:128], warm8[:, :, 0:256],
                start=(wi == 0), stop=(wi == NW - 1), perf_mode=DR,
            )

        # Gram accumulators: one full PSUM bank per 128-row block of G.
        g_ps = [ps.tile([128, C], F32, name="ps", tag="ps") for _ in range(CT)]

        xr = x.rearrange("(p t) c -> p t c", t=NT)

        # First groups small so PE work starts early, then larger groups.
        load_groups = [1, 1, 2, 4, 8, 8, 8]
        assert sum(load_groups) == NT
        k0 = 0
        for gi, gsz in enumerate(load_groups):
            nc.sync.dma_start(A32[:, k0:k0 + gsz, :], xr[:, k0:k0 + gsz, :])
            if gi == 0:
                # gamma: tiny load on the ACT HWDGE ring, off the input path
                nc.scalar.dma_start(gB[:], gamma[:])
            for j in range(gsz):
                k = k0 + j
                # cast f32 -> fp8 (DVE)
                nc.vector.tensor_copy(A8[:, k, :], A32[:, k, :])
                if k % 2 == 1:
                    kk = k - 1
                    # Gram first: one DoubleRow matmul per row-block per
                    # chunk pair; the last of these gates softmax.
                    for mi in range(CT):
                        nc.tensor.matmul(
                            g_ps[mi][:],
                            A8[:, kk:kk + 2, mi * 128:(mi + 1) * 128],
                            A8[:, kk:kk + 2, :],
                            start=(kk == 0), stop=(kk == NT - 2),
                            perf_mode=DR,
                        )
                    # then A^T blocks for both chunks of the pair: 8 PE
                    # transposes into one 2KB PSUM tile, one u16 copy out
                    tp = tps.tile([128, 2, CT, 128, 2], FP8,
                                  name="tp", tag="tp")
                    for j2 in range(2):
                        for ci in range(CT):
                            nc.tensor.transpose(
                                tp[:, j2, ci, :, 0],
                                A8[:, kk + j2, ci * 128:(ci + 1) * 128],
                                ident8[:],
                            )
                    nc.scalar.copy(
                        AT8[:, :, kk:kk + 2, :, :]
                        .rearrange("p ci k j two -> p k ci j two")
                        .bitcast(U16),
                        tp[:].bitcast(U16),
                    )
            k0 += gsz

        # softmax over rows of G (free axis), straight from PSUM
        for mi in range(CT):
            nmax = stat.tile([128, 1], F32)
            nc.vector.tensor_reduce(
                nmax[:], g_ps[mi][:],
                axis=mybir.AxisListType.X, op=mybir.AluOpType.max, negate=True,
            )
            esum = stat.tile([128, 1], F32)
            nc.scalar.activation(
                E32[:, mi, :], g_ps[mi][:],
                mybir.ActivationFunctionType.Exp,
                bias=nmax[:], scale=1.0, accum_out=esum[:],
            )
            rsum = stat.tile([128, 1], F32)
            nc.vector.reciprocal(rsum[:], esum[:])
            nc.vector.tensor_scalar_mul(P8[:, mi, :], E32[:, mi, :], rsum[:])

        # Y = A @ P (DoubleRow, 2 matmuls/chunk), epilogue gamma*Y + x.
        # y tiles rotate through the shared 6-buffer PSUM pool (recycling
        # the warm + Gram banks) so the PE runs well ahead of the DVE
        # epilogue drain.
        out_r = out.rearrange("(p t) c -> p t c", t=NT)
        out_groups = [4] * 7 + [2, 2]
        assert sum(out_groups) == NT
        t0 = 0
        for h, osz in enumerate(out_groups):
            o32 = ostage.tile([128, 4, C], F32)
            for j in range(osz):
                t = t0 + j
                y = ps.tile([128, C], F32, name="ps", tag="ps")
                for cp in range(CT // 2):
                    nc.tensor.matmul(
                        y[:],
                        AT8[:, 2 * cp:2 * cp + 2, t, :, 0],
                        P8[:, 2 * cp:2 * cp + 2, :],
                        start=(cp == 0), stop=(cp == CT // 2 - 1),
                        perf_mode=DR,
                    )
                nc.vector.scalar_tensor_tensor(
                    o32[:, j, :], y[:], gB[:], A32[:, t, :],
                    op0=mybir.AluOpType.mult, op1=mybir.AluOpType.add,
                )
            # last groups ride the idle ACT ring to dodge Sync-ring backlog
            oeng = nc.scalar if h >= len(out_groups) - 2 else nc.sync
            oeng.dma_start(out_r[:, t0:t0 + osz, :], o32[:, 0:osz, :])
            t0 += osz


def build():
    nc = bacc.Bacc("TRN2", target_bir_lowering=False, debug=False)
    x = nc.dram_tensor("x", [HW, C], F32, kind="ExternalInput").ap()
    gamma = nc.dram_tensor("gamma", [128, 1], F32, kind="ExternalInput").ap()
    out = nc.dram_tensor("out", [HW, C], F32, kind="ExternalOutput").ap()
    with tile.TileContext(nc) as tc:
        _emit(nc, tc, out, x, gamma)
    nc.compile()
    return nc


def kernel(x: np.ndarray, gamma: np.ndarray, trace: bool = False):
    assert x.shape == (B, H, W, C), x.shape
    if "nc" not in _CACHE:
        _CACHE["nc"] = build()
    nc = _CACHE["nc"]

    g128 = np.full((128, 1), np.float32(np.asarray(gamma).reshape(-1)[0]),
                   dtype=np.float32)
    in_maps = [
        {
            "x": np.ascontiguousarray(
                np.asarray(x[i], dtype=np.float32).reshape(HW, C)),
            "gamma": g128,
        }
        for i in range(B)
    ]
    if trace:
        res = run_bass_kernel_spmd(nc, in_maps, core_ids=list(range(B)),
                                   trace=True)
    else:
        # Force-untraced: a stray BASS_TRACE in the environment would route
        # through profiling hooks this image may not have.
        import os
        prev = os.environ.get("BASS_NEVER_TRACE")
        os.environ["BASS_NEVER_TRACE"] = "1"
        try:
            res = run_bass_kernel_spmd(nc, in_maps, core_ids=list(range(B)))
        finally:
            if prev is None:
                os.environ.pop("BASS_NEVER_TRACE", None)
            else:
                os.environ["BASS_NEVER_TRACE"] = prev
    _CACHE["last_result"] = res
    out = np.stack([res.results[i]["out"] for i in range(B)], axis=0)
    return out.reshape(B, H, W, C).astype(np.float32)
